# revision 1
# baseline (speedup 1.0000x reference)
"""ContactMapHead bilinear pair-scoring kernel for 8 trn2 NeuronCores.

Math: for each batch b, logits[b, p] = h[b, i_p] @ W @ h[b, j_p] + bias,
where (i_p, j_p) enumerate position pairs (upper triangle, k=1, when the
masks keep every position — the general case is handled too).

This equals S_b = (h_b @ W) @ h_b^T + bias followed by a pair gather.
S_b is a 512x512 matrix per batch; total device work = two 512^3 matmuls
per batch (memory-bound at this size).

Sharding (8 cores): core c computes rows [r0, r0+128) of S_b for batch
b = c // 4, r0 = (c % 4) * 128.  W, hs_b^T are replicated per core; each
core also gets its own pre-transposed row-slice hs_rows^T.  The host
assembles S (2, 512, 512) from the 8 row-blocks and gathers the pair
indices (pure unshard/reindex).

Device layout (per core), P = 128 partitions, all fp32:
  inputs  w    (512, 512)  W, row-major (k on partitions per 128-chunk)
          hst  (512, 512)  hs_b^T  (hst[k, j] = hs[b, j, k])
          hsrt (512, 128)  hs_rows^T (columns r0..r0+127 of hst)
          bias (1,)
  stage 1 G^T chunks: GT[hc] (128h x 128m) = sum_kc W[kc,hc-slice]^T-mm
          via matmul(lhsT=w[kc][:, hc], rhs=hsrt[kc])   (16 matmuls N=128)
  stage 2 S (128m x 512j) = sum_hc matmul(lhsT=GT[hc], rhs=hst[hc])
          (4 matmuls N=512, one PSUM bank)
  out     out (128, 512) = S + bias
"""

import numpy as np

_B, _L, _H = 2, 512, 512
_P = 128
_KC = _H // _P          # 4 contraction chunks
_GROUPS = 4             # row-blocks per batch
_RB = _L // _GROUPS     # 128 rows per core
_NCORES = 8

# Dev/profiling knobs (used by test.py only; harness leaves them alone).
TRACE = False
TRACE_KWARGS = {}
LAST_RESULTS = None

_STATE = {}


def _build_nc():
    """Build (once) the Bass module for one core's row-block computation."""
    if "nc" in _STATE:
        return _STATE["nc"]

    from concourse import bacc, mybir, tile

    f32 = mybir.dt.float32
    nc = bacc.Bacc(
        "TRN2", target_bir_lowering=False, debug=False, num_devices=_NCORES
    )

    w_d = nc.dram_tensor("w", [_H, _H], f32, kind="ExternalInput")
    hst_d = nc.dram_tensor("hst", [_H, _L], f32, kind="ExternalInput")
    hsrt_d = nc.dram_tensor("hsrt", [_H, _RB], f32, kind="ExternalInput")
    bias_d = nc.dram_tensor("bias", [1], f32, kind="ExternalInput")
    out_d = nc.dram_tensor("out", [_RB, _L], f32, kind="ExternalOutput")

    with tile.TileContext(nc) as tc:
        with (
            tc.tile_pool(name="sb", bufs=1) as sb,
            tc.tile_pool(name="ps", bufs=1, space="PSUM") as ps,
        ):
            bias_sb = sb.tile([_P, 1], f32, tag="bias")
            nc.sync.dma_start(bias_sb[:], bias_d[:].to_broadcast((_P, 1)))

            # hs_rows^T: one 256KB DMA, 3D tile indexed by k-chunk.
            hsrt_sb = sb.tile([_P, _KC, _RB], f32, tag="hsrt")
            nc.sync.dma_start(
                hsrt_sb[:], hsrt_d[:].rearrange("(o p) m -> p o m", p=_P)
            )

            # W: one 1MB DMA.
            w_sb = sb.tile([_P, _KC, _H], f32, tag="w")
            nc.sync.dma_start(w_sb[:], w_d[:].rearrange("(o p) h -> p o h", p=_P))

            # hs_b^T: 4 chunked DMAs so stage-2 matmuls chase arrivals.
            hst_sb = []
            for hc in range(_KC):
                t = sb.tile([_P, _L], f32, tag=f"hst{hc}")
                nc.sync.dma_start(t[:], hst_d[hc * _P : (hc + 1) * _P, :])
                hst_sb.append(t)

            # Stage 1: G^T chunks (each its own PSUM bank).
            gt_sb = sb.tile([_P, _H], f32, tag="gt_sb")
            for hc in range(_KC):
                pgt = ps.tile([_P, _P], f32, tag=f"gt{hc}")
                for kc in range(_KC):
                    nc.tensor.matmul(
                        pgt[:],
                        lhsT=w_sb[:, kc, hc * _P : (hc + 1) * _P],
                        rhs=hsrt_sb[:, kc, :],
                        start=(kc == 0),
                        stop=(kc == _KC - 1),
                    )
                nc.vector.tensor_copy(gt_sb[:, hc * _P : (hc + 1) * _P], pgt[:])

            # Stage 2: S rows, accumulated over h-chunks into one bank.
            psum_s = ps.tile([_P, _L], f32, tag="s")
            for hc in range(_KC):
                nc.tensor.matmul(
                    psum_s[:],
                    lhsT=gt_sb[:, hc * _P : (hc + 1) * _P],
                    rhs=hst_sb[hc][:],
                    start=(hc == 0),
                    stop=(hc == _KC - 1),
                )

            # Bias add + PSUM->SBUF, then store.
            out_sb = sb.tile([_P, _L], f32, tag="out_sb")
            nc.vector.tensor_scalar_add(out_sb[:], psum_s[:], bias_sb[:, 0:1])
            nc.sync.dma_start(out_d[:], out_sb[:])

    nc.compile()
    _STATE["nc"] = nc
    return nc


def _device_scores(hs, w, bias):
    """Compute S[b, i, j] = (hs_b @ W @ hs_b^T)[i, j] + bias on 8 cores."""
    global LAST_RESULTS
    from concourse.bass_utils import run_bass_kernel_spmd

    nc = _build_nc()

    hst = [np.ascontiguousarray(hs[b].T) for b in range(_B)]
    in_maps = []
    for c in range(_NCORES):
        b, rc = divmod(c, _GROUPS)
        r0 = rc * _RB
        in_maps.append(
            {
                "w": w,
                "hst": hst[b],
                "hsrt": np.ascontiguousarray(hst[b][:, r0 : r0 + _RB]),
                "bias": bias,
            }
        )

    kwargs = dict(TRACE_KWARGS) if TRACE else {}
    res = run_bass_kernel_spmd(
        nc, in_maps, core_ids=list(range(_NCORES)), trace=TRACE, **kwargs
    )
    LAST_RESULTS = res

    s = np.empty((_B, _L, _L), np.float32)
    for c in range(_NCORES):
        b, rc = divmod(c, _GROUPS)
        s[b, rc * _RB : (rc + 1) * _RB, :] = res.results[c]["out"]
    return s


def kernel(hidden_states, W, b, attention_mask, special_tokens_mask):
    hs = np.ascontiguousarray(np.asarray(hidden_states, dtype=np.float32))
    w = np.ascontiguousarray(np.asarray(W, dtype=np.float32)[0])
    bias = np.asarray(b, dtype=np.float32).reshape(1)
    am = np.asarray(attention_mask)
    sm = np.asarray(special_tokens_mask)

    # Pair indices from the (constant) masks — mirrors the reference.
    aa_mask = (am[0] == 1) & (sm[0] == 0)
    aa_positions = np.nonzero(aa_mask)[0]
    n_aa = aa_positions.shape[0]
    if n_aa < 2:
        return np.zeros((hs.shape[0], 0), dtype=np.float32)
    tri_i, tri_j = np.triu_indices(n_aa, k=1)
    idx_i = aa_positions[tri_i]
    idx_j = aa_positions[tri_j]

    if hs.shape != (_B, _L, _H) or w.shape != (_H, _H):
        # Defensive fallback for unexpected shapes (never hit by the spec).
        g = hs @ w
        s = np.einsum("bik,bjk->bij", g, hs) + bias[0]
        return s[:, idx_i, idx_j].astype(np.float32)

    s = _device_scores(hs, w, bias)  # bias already added on device
    return s[:, idx_i, idx_j].astype(np.float32)


# revision 7
# speedup vs baseline: 1.0689x; 1.0689x over previous
"""ContactMapHead bilinear pair-scoring kernel for 8 trn2 NeuronCores.

Math: for each batch b, logits[b, p] = h[b, i_p] @ W @ h[b, j_p] + bias,
where (i_p, j_p) enumerate position pairs (upper triangle, k=1, when the
masks keep every position — the general case is handled too).

This equals S_b = (h_b @ W) @ h_b^T + bias followed by a pair gather.
S_b is a 512x512 matrix per batch; total device work = two 512^3 matmuls
per batch (memory-bound at this size).

Sharding (8 cores): core c computes rows [r0, r0+128) of S_b for batch
b = c // 4, r0 = (c % 4) * 128.  W and hs_b^T are replicated per core;
each core also gets its own pre-transposed row-slice hs_rows^T.  The
host assembles S (2, 512, 512) from the 8 row-blocks and gathers the
pair indices (pure unshard/reindex).

Device program (per core), P = 128 partitions, all fp32, raw bass
(manual semaphores, no Tile entry/exit barriers).  All DRAM inputs are
host-swizzled to partition-major (128, X) so every DMA descriptor is a
large contiguous run:
    w    (128, 2048): w[p, kc*512 + :] = W[kc*128 + p, :]
    hst  (128, 2048): hst[p, hc*512 + j] = hs[b, j, hc*128 + p]
    hsrt (128, 512):  hsrt[p, kc*128 + m] = hs[b, r0 + m, kc*128 + p]
    bias (1,)
    out  (128, 512):  S rows r0..r0+127 (+bias)

  stage 1 (PE): GT[hc] (128h x 128m) += lhsT=W[kc, hc-cols] x rhs=hsrt[kc]
  copy  (DVE): gt_sb[:, hc] <- GT[hc]
  stage 2 (PE): ps (128m x 512j) += lhsT=gt_sb[:, hc] x rhs=hst[hc]
  epilogue (DVE+DMA): out = ps + bias, in two column halves on two rings
"""

import numpy as np

_B, _L, _H = 2, 512, 512
_P = 128
_KC = _H // _P          # 4 contraction chunks
_GROUPS = 4             # row-blocks per batch
_RB = _L // _GROUPS     # 128 rows per core
_NCORES = 8

# Dev/profiling knobs (used by test.py only; harness leaves them alone).
TRACE = False
TRACE_KWARGS = {}
LAST_RESULTS = None

_STATE = {}


def _build_nc():
    """Build (once) the raw-bass module for one core's row-block."""
    if "nc" in _STATE:
        return _STATE["nc"]

    from concourse import bacc, mybir

    f32 = mybir.dt.float32
    nc = bacc.Bacc("TRN2", target_bir_lowering=False, debug=False)

    w_d = nc.dram_tensor("w", [_P, 2048], f32, kind="ExternalInput")
    hst_d = nc.dram_tensor("hst", [_P, 2048], f32, kind="ExternalInput")
    hsrt_d = nc.dram_tensor("hsrt", [_P, 512], f32, kind="ExternalInput")
    bias_d = nc.dram_tensor("bias", [1], f32, kind="ExternalInput")
    out_d = nc.dram_tensor("out", [_RB, _L], f32, kind="ExternalOutput")

    w_sb = nc.alloc_sbuf_tensor("w_sb", [_P, 2048], f32)
    hst_sb = nc.alloc_sbuf_tensor("hst_sb", [_P, 2048], f32)
    hsrt_sb = nc.alloc_sbuf_tensor("hsrt_sb", [_P, 512], f32)
    bias_sb = nc.alloc_sbuf_tensor("bias_sb", [_P, 1], f32)
    gt_sb = nc.alloc_sbuf_tensor("gt_sb", [_P, 512], f32)
    out_sb = nc.alloc_sbuf_tensor("out_sb", [_P, _L], f32)
    pgt = [nc.alloc_psum_tensor(f"pgt{h}", [_P, _P], f32) for h in range(_KC)]
    ps = nc.alloc_psum_tensor("ps", [_P, _L], f32)

    s_w = [nc.alloc_semaphore(f"s_w{k}") for k in range(_KC)]  # +16 each
    s_hr = nc.alloc_semaphore("s_hr")      # +16 hsrt
    s_hst = [nc.alloc_semaphore(f"s_hst{h}") for h in range(_KC)]  # +16 each
    s_bias = nc.alloc_semaphore("s_bias")  # +16 bias
    s_gt_pe = nc.alloc_semaphore("s_gt_pe")  # +1 per stage-1 group
    s_gt_v = nc.alloc_semaphore("s_gt_v")    # +1 per gt copy
    s_s = nc.alloc_semaphore("s_s")        # +1 stage-2 done
    s_out = nc.alloc_semaphore("s_out")    # +1 per epilogue half
    s_od = nc.alloc_semaphore("s_od")      # +16 per out-DMA half

    HALF = _L // 2

    with nc.Block(no_gpsimd_drain=True) as block:

        @block.sync
        def _(sync):
            # critical inputs for stage 1 first
            sync.dma_start(out=hsrt_sb[:], in_=hsrt_d[:]).then_inc(s_hr, 16)
            sync.dma_start(out=w_sb[:, 0:512], in_=w_d[:, 0:512]).then_inc(
                s_w[0], 16
            )
            sync.dma_start(out=w_sb[:, 512:1024], in_=w_d[:, 512:1024]).then_inc(
                s_w[1], 16
            )
            sync.dma_start(
                out=bias_sb[:], in_=bias_d[:].to_broadcast((_P, 1))
            ).then_inc(s_bias, 16)
            sync.wait_ge(s_out, 1)
            sync.dma_start(out=out_d[:, 0:HALF], in_=out_sb[:, 0:HALF]).then_inc(
                s_od, 16
            )
            sync.wait_ge(s_od, 32)

        @block.scalar
        def _(scalar):
            scalar.dma_start(out=w_sb[:, 1024:1536], in_=w_d[:, 1024:1536]).then_inc(
                s_w[2], 16
            )
            scalar.dma_start(out=w_sb[:, 1536:2048], in_=w_d[:, 1536:2048]).then_inc(
                s_w[3], 16
            )
            for hc in range(_KC):
                scalar.dma_start(
                    out=hst_sb[:, hc * 512 : (hc + 1) * 512],
                    in_=hst_d[:, hc * 512 : (hc + 1) * 512],
                ).then_inc(s_hst[hc], 16)
            scalar.wait_ge(s_out, 2)
            scalar.dma_start(
                out=out_d[:, HALF:_L], in_=out_sb[:, HALF:_L]
            ).then_inc(s_od, 16)
            scalar.wait_ge(s_od, 32)

        @block.tensor
        def _(tensor):
            # kc-outer so round kc only needs W chunk kc (chases the DMAs)
            tensor.wait_ge(s_hr, 16)
            for kc in range(_KC):
                tensor.wait_ge(s_w[kc], 16)
                for hc in range(_KC):
                    mm = nc.tensor.matmul(
                        pgt[hc][:],
                        lhsT=w_sb[:, kc * 512 + hc * _P : kc * 512 + (hc + 1) * _P],
                        rhs=hsrt_sb[:, kc * _P : (kc + 1) * _P],
                        start=(kc == 0),
                        stop=(kc == _KC - 1),
                    )
                    if kc == _KC - 1:
                        mm.then_inc(s_gt_pe, 1)
            for hc in range(_KC):
                tensor.wait_ge(s_gt_v, hc + 1)
                tensor.wait_ge(s_hst[hc], 16)
                mm = nc.tensor.matmul(
                    ps[:],
                    lhsT=gt_sb[:, hc * _P : (hc + 1) * _P],
                    rhs=hst_sb[:, hc * 512 : (hc + 1) * 512],
                    start=(hc == 0),
                    stop=(hc == _KC - 1),
                )
            mm.then_inc(s_s, 1)

        @block.vector
        def _(vector):
            for hc in range(_KC):
                vector.wait_ge(s_gt_pe, hc + 1)
                nc.vector.tensor_copy(
                    gt_sb[:, hc * _P : (hc + 1) * _P], pgt[hc][:]
                ).then_inc(s_gt_v, 1)
            vector.wait_ge(s_s, 1)
            vector.wait_ge(s_bias, 16)
            nc.vector.tensor_scalar_add(
                out_sb[:, 0:HALF], ps[:, 0:HALF], bias_sb[:, 0:1]
            ).then_inc(s_out, 1)
            nc.vector.tensor_scalar_add(
                out_sb[:, HALF:_L], ps[:, HALF:_L], bias_sb[:, 0:1]
            ).then_inc(s_out, 1)

    nc.compile()
    _STATE["nc"] = nc
    return nc


def _swizzle(a):
    """(512, X) row-major -> (128, 4*X): partition p gets rows p, 128+p, ..."""
    x = a.shape[1]
    return np.ascontiguousarray(
        a.reshape(_KC, _P, x).transpose(1, 0, 2).reshape(_P, _KC * x)
    )


def _device_scores(hs, w, bias):
    """Compute S[b, i, j] = (hs_b @ W @ hs_b^T)[i, j] + bias on 8 cores."""
    global LAST_RESULTS
    from concourse.bass_utils import run_bass_kernel_spmd

    nc = _build_nc()

    w_p = _swizzle(w)
    hst = [np.ascontiguousarray(hs[b].T) for b in range(_B)]
    hst_p = [_swizzle(h) for h in hst]
    in_maps = []
    for c in range(_NCORES):
        b, rc = divmod(c, _GROUPS)
        r0 = rc * _RB
        in_maps.append(
            {
                "w": w_p,
                "hst": hst_p[b],
                "hsrt": _swizzle(hst[b][:, r0 : r0 + _RB]),
                "bias": bias,
            }
        )

    kwargs = dict(TRACE_KWARGS) if TRACE else {}
    res = run_bass_kernel_spmd(
        nc, in_maps, core_ids=list(range(_NCORES)), trace=TRACE, **kwargs
    )
    LAST_RESULTS = res

    s = np.empty((_B, _L, _L), np.float32)
    for c in range(_NCORES):
        b, rc = divmod(c, _GROUPS)
        s[b, rc * _RB : (rc + 1) * _RB, :] = res.results[c]["out"]
    return s


def kernel(hidden_states, W, b, attention_mask, special_tokens_mask):
    hs = np.ascontiguousarray(np.asarray(hidden_states, dtype=np.float32))
    w = np.ascontiguousarray(np.asarray(W, dtype=np.float32)[0])
    bias = np.asarray(b, dtype=np.float32).reshape(1)
    am = np.asarray(attention_mask)
    sm = np.asarray(special_tokens_mask)

    # Pair indices from the (constant) masks — mirrors the reference.
    aa_mask = (am[0] == 1) & (sm[0] == 0)
    aa_positions = np.nonzero(aa_mask)[0]
    n_aa = aa_positions.shape[0]
    if n_aa < 2:
        return np.zeros((hs.shape[0], 0), dtype=np.float32)
    tri_i, tri_j = np.triu_indices(n_aa, k=1)
    idx_i = aa_positions[tri_i]
    idx_j = aa_positions[tri_j]

    if hs.shape != (_B, _L, _H) or w.shape != (_H, _H):
        # Defensive fallback for unexpected shapes (never hit by the spec).
        g = hs @ w
        s = np.einsum("bik,bjk->bij", g, hs) + bias[0]
        return s[:, idx_i, idx_j].astype(np.float32)

    s = _device_scores(hs, w, bias)  # bias already added on device
    return s[:, idx_i, idx_j].astype(np.float32)


# revision 11
# speedup vs baseline: 1.1794x; 1.1034x over previous
"""ContactMapHead bilinear pair-scoring kernel for 8 trn2 NeuronCores.

Math: for each batch b, logits[b, p] = h[b, i_p] @ W @ h[b, j_p] + bias,
where (i_p, j_p) enumerate position pairs (upper triangle, k=1, when the
masks keep every position — the general case is handled too).

This equals S_b = (h_b @ W) @ h_b^T + bias followed by a pair gather.
S_b is a 512x512 matrix per batch; total device work = two 512^3 matmuls
per batch (memory-bound at this size).

Sharding (8 cores): core c computes rows [r0, r0+128) of S_b for batch
b = c // 4, r0 = (c % 4) * 128.  W and hs_b^T are replicated per core;
each core also gets its own pre-transposed row-slice hs_rows^T.  The
host assembles S (2, 512, 512) from the 8 row-blocks and gathers the
pair indices (pure unshard/reindex).

Device program (per core), P = 128 partitions, all fp32, raw bass
(manual semaphores, no Tile entry/exit barriers).  All DRAM inputs are
host-swizzled to partition-major (128, X) so every DMA descriptor is a
large contiguous run:
    w    (128, 2048): w[p, kc*512 + :] = W[kc*128 + p, :]
    hst  (128, 2048): hst[p, hc*512 + j] = hs[b, j, hc*128 + p]
    hsrt (128, 512):  hsrt[p, kc*128 + m] = hs[b, r0 + m, kc*128 + p]
    bias (1,)
    out  (128, 512):  S rows r0..r0+127 (+bias)

  stage 1 (PE): GT[hc] (128h x 128m) += lhsT=W[kc, hc-cols] x rhs=hsrt[kc]
  copy  (DVE): gt_sb[:, hc] <- GT[hc]
  stage 2 (PE): ps (128m x 512j) += lhsT=gt_sb[:, hc] x rhs=hst[hc]
  epilogue (DVE+DMA): out = ps + bias, in two column halves on two rings
"""

import numpy as np

_B, _L, _H = 2, 512, 512
_P = 128
_KC = _H // _P          # 4 contraction chunks
_GROUPS = 4             # row-blocks per batch
_RB = _L // _GROUPS     # 128 rows per core
_NCORES = 8

# Dev/profiling knobs (used by test.py only; harness leaves them alone).
TRACE = False
TRACE_KWARGS = {}
LAST_RESULTS = None

_STATE = {}


def _build_nc():
    """Build (once) the raw-bass module for one core's row-block."""
    if "nc" in _STATE:
        return _STATE["nc"]

    from concourse import bacc, mybir

    f32 = mybir.dt.float32
    nc = bacc.Bacc("TRN2", target_bir_lowering=False, debug=False)

    w_d = nc.dram_tensor("w", [_P, 2048], f32, kind="ExternalInput")
    hst_d = nc.dram_tensor("hst", [_P, 2048], f32, kind="ExternalInput")
    hsrt_d = nc.dram_tensor("hsrt", [_P, 512], f32, kind="ExternalInput")
    bias_d = nc.dram_tensor("bias", [1], f32, kind="ExternalInput")
    out_d = nc.dram_tensor("out", [_RB, _L], f32, kind="ExternalOutput")

    w_sb = nc.alloc_sbuf_tensor("w_sb", [_P, 2048], f32)
    hst_sb = nc.alloc_sbuf_tensor("hst_sb", [_P, 2048], f32)
    hsrt_sb = nc.alloc_sbuf_tensor("hsrt_sb", [_P, 512], f32)
    bias_sb = nc.alloc_sbuf_tensor("bias_sb", [_P, 1], f32)
    gt_sb = nc.alloc_sbuf_tensor("gt_sb", [_P, 512], f32)
    out_sb = nc.alloc_sbuf_tensor("out_sb", [_P, _L], f32)
    warm_sb = nc.alloc_sbuf_tensor("warm_sb", [_P, 512], f32)
    pgt = [nc.alloc_psum_tensor(f"pgt{h}", [_P, _P], f32) for h in range(_KC)]
    ps = nc.alloc_psum_tensor("ps", [_P, _L], f32)
    pwarm = nc.alloc_psum_tensor("pwarm", [_P, _L], f32)

    s_w = [nc.alloc_semaphore(f"s_w{k}") for k in range(_KC)]  # +16 each
    s_hr = nc.alloc_semaphore("s_hr")      # +16 hsrt
    s_hst = [nc.alloc_semaphore(f"s_hst{h}") for h in range(_KC)]  # +16 each
    s_bias = nc.alloc_semaphore("s_bias")  # +16 bias
    s_gt_pe = nc.alloc_semaphore("s_gt_pe")  # +1 per stage-1 group
    s_gt_v = nc.alloc_semaphore("s_gt_v")    # +1 per gt copy
    s_s = nc.alloc_semaphore("s_s")        # +1 stage-2 done
    s_out = nc.alloc_semaphore("s_out")    # +1 per epilogue half
    s_od = nc.alloc_semaphore("s_od")      # +16 per out-DMA half
    s_warm = nc.alloc_semaphore("s_warm")  # +1 warmup scratch zeroed

    HALF = _L // 2

    with nc.Block(no_gpsimd_drain=True) as block:

        @block.sync
        def _(sync):
            # critical inputs for stage 1 first
            sync.dma_start(out=hsrt_sb[:], in_=hsrt_d[:]).then_inc(s_hr, 16)
            sync.dma_start(out=w_sb[:, 0:512], in_=w_d[:, 0:512]).then_inc(
                s_w[0], 16
            )
            sync.dma_start(out=w_sb[:, 512:1024], in_=w_d[:, 512:1024]).then_inc(
                s_w[1], 16
            )
            sync.dma_start(
                out=bias_sb[:], in_=bias_d[:].to_broadcast((_P, 1))
            ).then_inc(s_bias, 16)
            sync.wait_ge(s_out, 1)
            sync.dma_start(out=out_d[:, 0:HALF], in_=out_sb[:, 0:HALF]).then_inc(
                s_od, 16
            )
            sync.wait_ge(s_od, 32)

        @block.scalar
        def _(scalar):
            scalar.dma_start(out=w_sb[:, 1024:1536], in_=w_d[:, 1024:1536]).then_inc(
                s_w[2], 16
            )
            scalar.dma_start(out=w_sb[:, 1536:2048], in_=w_d[:, 1536:2048]).then_inc(
                s_w[3], 16
            )
            for hc in range(_KC):
                scalar.dma_start(
                    out=hst_sb[:, hc * 512 : (hc + 1) * 512],
                    in_=hst_d[:, hc * 512 : (hc + 1) * 512],
                ).then_inc(s_hst[hc], 16)
            scalar.wait_ge(s_out, 2)
            scalar.dma_start(
                out=out_d[:, HALF:_L], in_=out_sb[:, HALF:_L]
            ).then_inc(s_od, 16)
            scalar.wait_ge(s_od, 32)

        @block.tensor
        def _(tensor):
            # HAM warmup: keep the PE array busy on zeros so the clock gate
            # opens (1.2 -> 2.4 GHz) before the real matmuls arrive.
            tensor.wait_ge(s_warm, 1)
            for _ in range(6):
                nc.tensor.matmul(
                    pwarm[:],
                    lhsT=warm_sb[:, 0:_P],
                    rhs=warm_sb[:],
                    start=True,
                    stop=True,
                )
            # kc-outer so round kc only needs W chunk kc (chases the DMAs)
            tensor.wait_ge(s_hr, 16)
            for kc in range(_KC):
                tensor.wait_ge(s_w[kc], 16)
                for hc in range(_KC):
                    mm = nc.tensor.matmul(
                        pgt[hc][:],
                        lhsT=w_sb[:, kc * 512 + hc * _P : kc * 512 + (hc + 1) * _P],
                        rhs=hsrt_sb[:, kc * _P : (kc + 1) * _P],
                        start=(kc == 0),
                        stop=(kc == _KC - 1),
                    )
                    if kc == _KC - 1:
                        mm.then_inc(s_gt_pe, 1)
            for hc in range(_KC):
                tensor.wait_ge(s_gt_v, hc + 1)
                tensor.wait_ge(s_hst[hc], 16)
                mm = nc.tensor.matmul(
                    ps[:],
                    lhsT=gt_sb[:, hc * _P : (hc + 1) * _P],
                    rhs=hst_sb[:, hc * 512 : (hc + 1) * 512],
                    start=(hc == 0),
                    stop=(hc == _KC - 1),
                )
            mm.then_inc(s_s, 1)

        @block.vector
        def _(vector):
            nc.vector.memset(warm_sb[:], 0.0).then_inc(s_warm, 1)
            for hc in range(_KC):
                vector.wait_ge(s_gt_pe, hc + 1)
                nc.vector.tensor_copy(
                    gt_sb[:, hc * _P : (hc + 1) * _P], pgt[hc][:]
                ).then_inc(s_gt_v, 1)
            vector.wait_ge(s_s, 1)
            vector.wait_ge(s_bias, 16)
            nc.vector.tensor_scalar_add(
                out_sb[:, 0:HALF], ps[:, 0:HALF], bias_sb[:, 0:1]
            ).then_inc(s_out, 1)
            nc.vector.tensor_scalar_add(
                out_sb[:, HALF:_L], ps[:, HALF:_L], bias_sb[:, 0:1]
            ).then_inc(s_out, 1)

    nc.compile()
    _STATE["nc"] = nc
    return nc


def _swizzle(a):
    """(512, X) row-major -> (128, 4*X): partition p gets rows p, 128+p, ..."""
    x = a.shape[1]
    return np.ascontiguousarray(
        a.reshape(_KC, _P, x).transpose(1, 0, 2).reshape(_P, _KC * x)
    )


def _device_scores(hs, w, bias):
    """Compute S[b, i, j] = (hs_b @ W @ hs_b^T)[i, j] + bias on 8 cores."""
    global LAST_RESULTS
    from concourse.bass_utils import run_bass_kernel_spmd

    nc = _build_nc()

    w_p = _swizzle(w)
    hst = [np.ascontiguousarray(hs[b].T) for b in range(_B)]
    hst_p = [_swizzle(h) for h in hst]
    in_maps = []
    for c in range(_NCORES):
        b, rc = divmod(c, _GROUPS)
        r0 = rc * _RB
        in_maps.append(
            {
                "w": w_p,
                "hst": hst_p[b],
                "hsrt": _swizzle(hst[b][:, r0 : r0 + _RB]),
                "bias": bias,
            }
        )

    kwargs = dict(TRACE_KWARGS) if TRACE else {}
    res = run_bass_kernel_spmd(
        nc, in_maps, core_ids=list(range(_NCORES)), trace=TRACE, **kwargs
    )
    LAST_RESULTS = res

    s = np.empty((_B, _L, _L), np.float32)
    for c in range(_NCORES):
        b, rc = divmod(c, _GROUPS)
        s[b, rc * _RB : (rc + 1) * _RB, :] = res.results[c]["out"]
    return s


def kernel(hidden_states, W, b, attention_mask, special_tokens_mask):
    hs = np.ascontiguousarray(np.asarray(hidden_states, dtype=np.float32))
    w = np.ascontiguousarray(np.asarray(W, dtype=np.float32)[0])
    bias = np.asarray(b, dtype=np.float32).reshape(1)
    am = np.asarray(attention_mask)
    sm = np.asarray(special_tokens_mask)

    # Pair indices from the (constant) masks — mirrors the reference.
    aa_mask = (am[0] == 1) & (sm[0] == 0)
    aa_positions = np.nonzero(aa_mask)[0]
    n_aa = aa_positions.shape[0]
    if n_aa < 2:
        return np.zeros((hs.shape[0], 0), dtype=np.float32)
    tri_i, tri_j = np.triu_indices(n_aa, k=1)
    idx_i = aa_positions[tri_i]
    idx_j = aa_positions[tri_j]

    if hs.shape != (_B, _L, _H) or w.shape != (_H, _H):
        # Defensive fallback for unexpected shapes (never hit by the spec).
        g = hs @ w
        s = np.einsum("bik,bjk->bij", g, hs) + bias[0]
        return s[:, idx_i, idx_j].astype(np.float32)

    s = _device_scores(hs, w, bias)  # bias already added on device
    return s[:, idx_i, idx_j].astype(np.float32)
